# revision 12
# baseline (speedup 1.0000x reference)
"""Trainium2 Bass kernel for sliding-window (window=256) causal attention.

Model (B=1, S=4096, H=1024, nh=16, hd=64, no q-scaling):
  q,k,v = x@wq.T, x@wk.T, x@wv.T ; scores = q@k.T (banded causal window 256)
  out = softmax(scores)@v reassembled, then @wo.T + bo

Sharding: 2 heads per core across 8 cores (tensor parallel on the head dim).
Each core computes a partial output  ctx_c @ wo[:, c-slice].T  (+ bias on
core 0 only); the host sums the 8 bf16 partials (the all-reduce step).

v3 (fp32 q/k/scores for precision, bf16 attention weights + out path,
x pre-transposed on host, phases interleaved per 512-column chunk):
  chunk sc: DMA xT chunk (fp32r); QT/KT = w.T.T @ xT (fp32r, FD=512);
            VT likewise, then PE-transpose 128-tiles -> va bf16 padded
            [V_h0 | 0 | V_h1] (per k-tile [s, d] orientation).
  key-tile kt (4 per chunk, interleaved):
      scoresT[k,q] h0/h1 fp32r (row-tiled stationaries) into a 2-bank psum
      tile; band-mask added via fp32r identity-matmul of m3=[tri|0|tri];
      one exp over both heads (ACT) -> ex bf16;
      ctx/dn accumulated per q-tile with padded bf16 stationaries; ctx+dn
      of two consecutive q-tiles share one psum bank (single accumulation
      group per bank, per-element first-write-overwrite semantics);
      normalize with reciprocal+mul (DVE, direct from psum);
      out-projection (bf16, FD=512 x2 into one 2-bank psum tile) + bias:
      half0 fused into the DVE psum->sbuf add, half1 via ACT copy +
      GPSIMD bias add; DMA out bf16.

PSUM budget (8 banks): tsc pool [128,2,512] bufs=3 (6 banks; scores +
out-proj share) + tcd pool [128,512] bufs=2 (2 banks; q/k/v psums,
transpose psums, and ctx/dn pair banks share).
"""

import numpy as np
import ml_dtypes

import concourse.bass as bass
import concourse.tile as tile
from concourse import bacc, mybir
from concourse.bass_utils import run_bass_kernel_spmd

S = 4096
H = 1024
NH = 16
HD = 64
WIN = 256
N_CORES = 8
HEADS_PER_CORE = NH // N_CORES  # 2
CD = HEADS_PER_CORE * HD  # 128 ctx dims per core
NEG = -1e30

F32 = mybir.dt.float32
F32R = mybir.dt.float32r
BF16 = mybir.dt.bfloat16

N_ST = S // 128  # 32 s-tiles
N_KT = H // 128  # 8 contraction tiles for projections
N_SC = S // 512  # 8 s-chunks for projections
BF = ml_dtypes.bfloat16


def build_program(taps=False, reps=1):
    nc = bacc.Bacc("TRN2", target_bir_lowering=False, debug=False)

    xT_ap = nc.dram_tensor("xT_b", [128, N_KT, S], F32R, kind="ExternalInput").ap()
    wq_ap = nc.dram_tensor("wq_b", [128, N_KT, CD], F32R, kind="ExternalInput").ap()
    wk_ap = nc.dram_tensor("wk_b", [128, N_KT, CD], F32R, kind="ExternalInput").ap()
    wv_ap = nc.dram_tensor("wv_b", [128, N_KT, CD], F32R, kind="ExternalInput").ap()
    wo_ap = nc.dram_tensor("wo_b", [CD, H], BF16, kind="ExternalInput").ap()
    bo_ap = nc.dram_tensor("bo_b", [128, H], BF16, kind="ExternalInput").ap()
    m3_ap = nc.dram_tensor("m3_b", [128, 384], F32R, kind="ExternalInput").ap()
    id_ap = nc.dram_tensor("id_b", [128, 128], F32R, kind="ExternalInput").ap()
    out_ap = nc.dram_tensor("out", [S, H], BF16, kind="ExternalOutput").ap()

    with tile.TileContext(nc) as tc:
        with (
            tc.tile_pool(name="consts", bufs=1) as consts,
            tc.tile_pool(name="big", bufs=1) as big,
        ):
            # ---- constant loads ----
            wq_sb = consts.tile([128, N_KT, CD], F32R)
            wk_sb = consts.tile([128, N_KT, CD], F32R)
            wv_sb = consts.tile([128, N_KT, CD], F32R)
            for w_sb, w_ap in ((wq_sb, wq_ap), (wk_sb, wk_ap), (wv_sb, wv_ap)):
                nc.sync.dma_start(out=w_sb[:], in_=w_ap[:])
            wo_sb = consts.tile([128, H], BF16)
            nc.sync.dma_start(out=wo_sb[:], in_=wo_ap[:])
            bo_sb = consts.tile([128, H], BF16)
            nc.sync.dma_start(out=bo_sb[:], in_=bo_ap[:])
            m3_sb = consts.tile([128, 384], F32R)
            nc.sync.dma_start(out=m3_sb[:], in_=m3_ap[:])
            id_sb = consts.tile([128, 128], F32R)
            nc.sync.dma_start(out=id_sb[:], in_=id_ap[:])
            # dn stationaries: [ones | zeros | ones]; h0 -> 0:128, h1 -> 64:192
            on2 = consts.tile([128, 192], BF16)
            nc.gpsimd.memset(on2[:, 0:64], 1.0)
            nc.gpsimd.memset(on2[:, 64:128], 0.0)
            nc.gpsimd.memset(on2[:, 128:192], 1.0)

            # ---- persistent activations ----
            qt_sb = big.tile([128, S], F32R)  # QT: [2h*64 dims, S]
            kt_sb = big.tile([128, S], F32R)
            # V per k-tile, padded: [V_h0(64) | zeros(64) | V_h1(64)]
            va = big.tile([128, N_ST, 192], BF16)
            nc.gpsimd.memset(va[:, :, 64:128], 0.0)

            for _rep in range(reps):
                with (
                    tc.tile_pool(name="xst", bufs=2) as xst_p,
                    tc.tile_pool(name="vst", bufs=2) as vst_p,
                    tc.tile_pool(name="pssc", bufs=3, space="PSUM") as pssc,
                    tc.tile_pool(name="pscd", bufs=2, space="PSUM") as pscd,
                    tc.tile_pool(name="expp", bufs=3) as expp,
                    tc.tile_pool(name="recp", bufs=2) as recp,
                    tc.tile_pool(name="ctxp", bufs=3) as ctxp,
                    tc.tile_pool(name="outp", bufs=3) as outp,
                ):
                    cd_tiles = {}  # pair index -> psum tile [128, 512]
                    cp_state = [0]

                    def emit_chunk(sc):
                        s0 = sc * 512
                        xst = xst_p.tile([128, N_KT, 512], F32R, tag="xst", name="xst")
                        nc.sync.dma_start(out=xst[:], in_=xT_ap[:, :, s0 : s0 + 512])
                        for w_sb, dstT in ((wq_sb, qt_sb), (wk_sb, kt_sb)):
                            pps = pscd.tile([128, 512], F32, tag="tcd")
                            for kt in range(N_KT):
                                nc.tensor.matmul(
                                    pps[:],
                                    w_sb[:, kt, :],
                                    xst[:, kt, :],
                                    start=(kt == 0),
                                    stop=(kt == N_KT - 1),
                                )
                            dst = dstT[:, s0 : s0 + 512]
                            if cp_state[0] % 2 == 0:
                                nc.vector.tensor_copy(dst, pps[:])
                            else:
                                nc.scalar.copy(dst, pps[:])
                            cp_state[0] += 1
                        # V: VT chunk then PE-transpose each 128-tile
                        vps = pscd.tile([128, 512], F32, tag="tcd")
                        for kt in range(N_KT):
                            nc.tensor.matmul(
                                vps[:],
                                wv_sb[:, kt, :],
                                xst[:, kt, :],
                                start=(kt == 0),
                                stop=(kt == N_KT - 1),
                            )
                        vst = vst_p.tile([128, 512], F32R, tag="vst", name="vst")
                        if cp_state[0] % 2 == 0:
                            nc.vector.tensor_copy(vst[:], vps[:])
                        else:
                            nc.scalar.copy(vst[:], vps[:])
                        cp_state[0] += 1
                        for st4 in range(4):
                            st = sc * 4 + st4
                            tp = pscd.tile([128, 512], F32R, tag="tcd", name=f"tp{st}")
                            nc.tensor.transpose(
                                tp[:, 0:128],
                                vst[:, st4 * 128 : (st4 + 1) * 128],
                                id_sb[:],
                            )
                            nc.vector.tensor_copy(va[:, st, 0:64], tp[:, 0:64])
                            nc.scalar.copy(va[:, st, 128:192], tp[:, 64:128])

                    def emit_kt(kt):
                        nj = min(3, N_ST - kt)
                        W = nj * 128
                        q0 = kt * 128
                        # scoresT both heads (fp32r, row-tiled stationaries)
                        sps = pssc.tile([128, 2, 512], F32, tag="tsc")
                        for h in (0, 1):
                            nc.tensor.matmul(
                                sps[:, h, 0:W],
                                kt_sb[h * 64 : (h + 1) * 64, q0 : q0 + 128],
                                qt_sb[h * 64 : (h + 1) * 64, q0 : q0 + W],
                                start=True,
                                stop=False,
                            )
                        # band mask add: m3 = [tri_a | 0 | tri_b]
                        for h in (0, 1):
                            nc.tensor.matmul(
                                sps[:, h, 0:W],
                                id_sb[:],
                                m3_sb[:, 0:W],
                                start=False,
                                stop=True,
                            )
                        # exp over both heads in one ACT op
                        ex = expp.tile([128, 2, 384], BF16)
                        nc.scalar.activation(
                            ex[:, :, 0:W],
                            sps[:, :, 0:W],
                            mybir.ActivationFunctionType.Exp,
                        )
                        # ctx + dn; q-tile pair (2m, 2m+1) shares one bank:
                        # [ctx_even | dn_even | ctx_odd | dn_odd] x 128
                        for j in range(nj):
                            qt = kt + j
                            m = qt // 2
                            if m not in cd_tiles:
                                cd_tiles[m] = pscd.tile(
                                    [128, 512], F32, tag="tcd", name=f"cd{m}"
                                )
                        for grp in ("ctx", "dn"):
                            for j in range(nj):
                                qt = kt + j
                                m = qt // 2
                                off = (qt % 2) * 256
                                dofs = 0 if grp == "ctx" else 128
                                cd = cd_tiles[m]
                                first = (
                                    grp == "ctx"
                                    and qt == 2 * m
                                    and kt == max(2 * m - 2, 0)
                                )
                                last = grp == "dn" and qt == 2 * m + 1 and kt == qt
                                for h in (0, 1):
                                    if grp == "ctx":
                                        lhsT = va[:, kt, h * 64 : h * 64 + 128]
                                    else:
                                        lhsT = on2[:, h * 64 : h * 64 + 128]
                                    nc.tensor.matmul(
                                        cd[:, off + dofs : off + dofs + 128],
                                        lhsT,
                                        ex[:, h, j * 128 : (j + 1) * 128],
                                        start=(first and h == 0),
                                        stop=(last and h == 1),
                                        skip_group_check=True,
                                    )
                        # finalize q-tile qt == kt
                        qt = kt
                        m = qt // 2
                        off = (qt % 2) * 256
                        cd = cd_tiles[m]
                        rec = recp.tile([128, 128], F32)
                        nc.vector.reciprocal(rec[:], cd[:, off + 128 : off + 256])
                        ctxn = ctxp.tile([128, 128], BF16)
                        nc.vector.tensor_mul(ctxn[:], cd[:, off : off + 128], rec[:])
                        if qt % 2 == 1 or qt == N_ST - 1:
                            del cd_tiles[m]
                        # out-projection + bias; both halves in one psum tile
                        osb = outp.tile([128, H], BF16)
                        ops = pssc.tile([128, 2, 512], F32, tag="tsc")
                        nc.tensor.matmul(
                            ops[:, 0, :], ctxn[:], wo_sb[:, 0:512], start=True, stop=True
                        )
                        nc.tensor.matmul(
                            ops[:, 1, :],
                            ctxn[:],
                            wo_sb[:, 512:1024],
                            start=True,
                            stop=True,
                        )
                        nc.vector.tensor_add(osb[:, 0:512], ops[:, 0, :], bo_sb[:, 0:512])
                        nc.scalar.copy(osb[:, 512:1024], ops[:, 1, :])
                        nc.gpsimd.tensor_add(
                            osb[:, 512:1024], osb[:, 512:1024], bo_sb[:, 512:1024]
                        )
                        nc.sync.dma_start(
                            out=out_ap[qt * 128 : (qt + 1) * 128, :], in_=osb[:]
                        )

                    # interleaved emission: chunk sc, then kts 4sc-2..4sc+1
                    for sc in range(N_SC):
                        emit_chunk(sc)
                        for kt in range(max(4 * sc - 2, 0), 4 * sc + 2):
                            emit_kt(kt)
                    for kt in (N_ST - 2, N_ST - 1):
                        emit_kt(kt)

    nc.compile()
    return nc


def build_in_maps(x, wq, wk, wv, wo, bo):
    xf = np.asarray(x, dtype=np.float32).reshape(S, H)
    # xT blocked: (p, kt, s) = x[s, kt*128 + p]
    xT_b = np.ascontiguousarray(xf.reshape(S, N_KT, 128).transpose(2, 1, 0))

    b = np.arange(128)[:, None]  # k within tile
    a = np.arange(128)[None, :]  # q within tile
    mask_a = np.where(b <= a, 0.0, NEG).astype(np.float32)  # diag tile (qt==kt)
    mask_b = np.where(b > a, 0.0, NEG).astype(np.float32)  # qt==kt+2 tile
    m3 = np.ascontiguousarray(
        np.concatenate([mask_a, np.zeros((128, 128), np.float32), mask_b], axis=1)
    )
    ident = np.eye(128, dtype=np.float32)

    def blk(wT):  # [H, CD] -> [128, N_KT, CD] float32
        return np.ascontiguousarray(
            wT.reshape(N_KT, 128, CD).transpose(1, 0, 2).astype(np.float32)
        )

    in_maps = []
    for c in range(N_CORES):
        r0, r1 = c * CD, (c + 1) * CD
        bo_c = (bo if c == 0 else np.zeros_like(bo)).astype(np.float32)
        bo_b = np.ascontiguousarray(np.broadcast_to(bo_c, (128, H)).astype(BF))
        in_maps.append(
            {
                "xT_b": xT_b,
                "wq_b": blk(np.asarray(wq, np.float32)[r0:r1, :].T),
                "wk_b": blk(np.asarray(wk, np.float32)[r0:r1, :].T),
                "wv_b": blk(np.asarray(wv, np.float32)[r0:r1, :].T),
                "wo_b": np.ascontiguousarray(
                    np.asarray(wo, np.float32)[:, r0:r1].T.astype(BF)
                ),
                "bo_b": bo_b,
                "m3_b": m3,
                "id_b": ident,
            }
        )
    return in_maps


_NC_CACHE = None


def kernel(x, wq, wk, wv, wo, bo):
    global _NC_CACHE
    if _NC_CACHE is None:
        _NC_CACHE = build_program()
    nc = _NC_CACHE
    in_maps = build_in_maps(x, wq, wk, wv, wo, bo)
    res = run_bass_kernel_spmd(nc, in_maps, list(range(N_CORES)))
    out = res.results[0]["out"].astype(np.float64)
    for c in range(1, N_CORES):
        out += res.results[c]["out"].astype(np.float64)
    return out.reshape(1, S, H).astype(np.float32)


# revision 23
# speedup vs baseline: 1.0903x; 1.0903x over previous
"""Trainium2 Bass kernel for sliding-window (window=256) causal attention.

Model (B=1, S=4096, H=1024, nh=16, hd=64, no q-scaling):
  q,k,v = x@wq.T, x@wk.T, x@wv.T ; scores = q@k.T (banded causal window 256)
  out = softmax(scores)@v reassembled, then @wo.T + bo

Sharding: 2 heads per core across 8 cores (tensor parallel on the head dim).
Each core computes a partial output  ctx_c @ wo[:, c-slice].T  (+ bias on
core 0 only); the host sums the 8 bf16 partials (the all-reduce step).

v3 (fp32 q/k/scores for precision, bf16 attention weights + out path,
x pre-transposed on host, phases interleaved per 512-column chunk):
  chunk sc: DMA xT chunk (fp32r); QT/KT = w.T.T @ xT (fp32r, FD=512);
            VT likewise, then PE-transpose 128-tiles -> va bf16 padded
            [V_h0 | 0 | V_h1] (per k-tile [s, d] orientation).
  key-tile kt (4 per chunk, interleaved):
      scoresT[k,q] h0/h1 fp32r (row-tiled stationaries) into a 2-bank psum
      tile; band-mask added via fp32r identity-matmul of m3=[tri|0|tri];
      one exp over both heads (ACT) -> ex bf16;
      ctx/dn accumulated per q-tile with padded bf16 stationaries; ctx+dn
      of two consecutive q-tiles share one psum bank (single accumulation
      group per bank, per-element first-write-overwrite semantics);
      normalize with reciprocal+mul (DVE, direct from psum);
      out-projection (bf16, FD=512 x2 into one 2-bank psum tile) + bias:
      half0 fused into the DVE psum->sbuf add, half1 via ACT copy +
      GPSIMD bias add; DMA out bf16.

PSUM budget (8 banks): tsc pool [128,2,512] bufs=3 (6 banks; scores +
out-proj share) + tcd pool [128,512] bufs=2 (2 banks; q/k/v psums,
transpose psums, and ctx/dn pair banks share).
"""

import numpy as np
import ml_dtypes

import concourse.bass as bass
import concourse.tile as tile
from concourse import bacc, mybir
from concourse.bass_utils import run_bass_kernel_spmd

S = 4096
H = 1024
NH = 16
HD = 64
WIN = 256
N_CORES = 8
HEADS_PER_CORE = NH // N_CORES  # 2
CD = HEADS_PER_CORE * HD  # 128 ctx dims per core
NEG = -1e30

F32 = mybir.dt.float32
F32R = mybir.dt.float32r
BF16 = mybir.dt.bfloat16

N_ST = S // 128  # 32 s-tiles
N_KT = H // 128  # 8 contraction tiles for projections
N_SC = S // 512  # 8 s-chunks for projections
BF = ml_dtypes.bfloat16


def build_program(taps=False, reps=1, chunk_in_tsc=False, out_q="sync",
                  tsc_bufs=2, tcd_bufs=4, xst_bufs=2, expp_bufs=3,
                  outp_bufs=3, ctxp_bufs=3):
    nc = bacc.Bacc("TRN2", target_bir_lowering=False, debug=False)

    xT_ap = nc.dram_tensor("xT_b", [128, N_KT, S], F32R, kind="ExternalInput").ap()
    wq_ap = nc.dram_tensor("wq_b", [128, N_KT, CD], F32R, kind="ExternalInput").ap()
    wk_ap = nc.dram_tensor("wk_b", [128, N_KT, CD], F32R, kind="ExternalInput").ap()
    wv_ap = nc.dram_tensor("wv_b", [128, N_KT, CD], F32R, kind="ExternalInput").ap()
    wo_ap = nc.dram_tensor("wo_b", [CD, H], BF16, kind="ExternalInput").ap()
    bo_ap = nc.dram_tensor("bo_b", [128, H], BF16, kind="ExternalInput").ap()
    m3_ap = nc.dram_tensor("m3_b", [128, 384], F32R, kind="ExternalInput").ap()
    id_ap = nc.dram_tensor("id_b", [128, 128], F32R, kind="ExternalInput").ap()
    out_ap = nc.dram_tensor("out", [S, H], BF16, kind="ExternalOutput").ap()

    with tile.TileContext(nc) as tc:
        with (
            tc.tile_pool(name="consts", bufs=1) as consts,
            tc.tile_pool(name="big", bufs=1) as big,
        ):
            # ---- constant loads ----
            wq_sb = consts.tile([128, N_KT, CD], F32R)
            wk_sb = consts.tile([128, N_KT, CD], F32R)
            wv_sb = consts.tile([128, N_KT, CD], F32R)
            for w_sb, w_ap in ((wq_sb, wq_ap), (wk_sb, wk_ap), (wv_sb, wv_ap)):
                nc.sync.dma_start(out=w_sb[:], in_=w_ap[:])
            wo_sb = consts.tile([128, H], BF16)
            nc.sync.dma_start(out=wo_sb[:], in_=wo_ap[:])
            bo_sb = consts.tile([128, H], BF16)
            nc.sync.dma_start(out=bo_sb[:], in_=bo_ap[:])
            m3_sb = consts.tile([128, 384], F32R)
            nc.sync.dma_start(out=m3_sb[:], in_=m3_ap[:])
            id_sb = consts.tile([128, 128], F32R)
            nc.sync.dma_start(out=id_sb[:], in_=id_ap[:])
            # dn stationaries: [ones | zeros | ones]; h0 -> 0:128, h1 -> 64:192
            on2 = consts.tile([128, 192], BF16)
            nc.gpsimd.memset(on2[:, 0:64], 1.0)
            nc.gpsimd.memset(on2[:, 64:128], 0.0)
            nc.gpsimd.memset(on2[:, 128:192], 1.0)

            # ---- persistent activations ----
            qt_sb = big.tile([128, S], F32R)  # QT: [2h*64 dims, S]
            kt_sb = big.tile([128, S], F32R)
            # V per k-tile, padded: [V_h0(64) | zeros(64) | V_h1(64)]
            va = big.tile([128, N_ST, 192], BF16)
            nc.gpsimd.memset(va[:, :, 64:128], 0.0)

            for _rep in range(reps):
                with (
                    tc.tile_pool(name="xst", bufs=xst_bufs) as xst_p,
                    tc.tile_pool(name="vst", bufs=2) as vst_p,
                    tc.tile_pool(name="pssc", bufs=tsc_bufs, space="PSUM") as pssc,
                    tc.tile_pool(name="pscd", bufs=tcd_bufs, space="PSUM") as pscd,
                    tc.tile_pool(name="expp", bufs=expp_bufs) as expp,
                    tc.tile_pool(name="recp", bufs=3) as recp,
                    tc.tile_pool(name="ctxp", bufs=ctxp_bufs) as ctxp,
                    tc.tile_pool(name="outp", bufs=outp_bufs) as outp,
                ):
                    cd_tiles = {}  # pair index -> psum tile [128, 512]
                    cp_state = [0]

                    def chunk_ps(name):
                        # [128,512] psum for phase-1; lives in the tsc pool
                        # (short-lived neighbors) or the tcd pool.
                        if chunk_in_tsc:
                            t = pssc.tile([128, 2, 512], F32, tag="tsc", name=name)
                            return t[:, 0, :]
                        return pscd.tile([128, 512], F32, tag="tcd", name=name)

                    def emit_chunk(sc):
                        s0 = sc * 512
                        xst = xst_p.tile([128, N_KT, 512], F32R, tag="xst", name="xst")
                        x_eng = nc.sync if sc % 2 == 0 else nc.scalar
                        x_eng.dma_start(out=xst[:], in_=xT_ap[:, :, s0 : s0 + 512])
                        for w_sb, dstT in ((wq_sb, qt_sb), (wk_sb, kt_sb)):
                            pps = chunk_ps("pps")
                            for kt in range(N_KT):
                                nc.tensor.matmul(
                                    pps[:],
                                    w_sb[:, kt, :],
                                    xst[:, kt, :],
                                    start=(kt == 0),
                                    stop=(kt == N_KT - 1),
                                )
                            dst = dstT[:, s0 : s0 + 512]
                            if cp_state[0] % 2 == 0:
                                nc.vector.tensor_copy(dst, pps[:])
                            else:
                                nc.scalar.copy(dst, pps[:])
                            cp_state[0] += 1
                        # V: VT chunk then PE-transpose each 128-tile
                        vps = chunk_ps("vps")
                        for kt in range(N_KT):
                            nc.tensor.matmul(
                                vps[:],
                                wv_sb[:, kt, :],
                                xst[:, kt, :],
                                start=(kt == 0),
                                stop=(kt == N_KT - 1),
                            )
                        vst = vst_p.tile([128, 512], F32R, tag="vst", name="vst")
                        if cp_state[0] % 2 == 0:
                            nc.vector.tensor_copy(vst[:], vps[:])
                        else:
                            nc.scalar.copy(vst[:], vps[:])
                        cp_state[0] += 1
                        for st4 in range(4):
                            st = sc * 4 + st4
                            tp = chunk_ps("tp").bitcast(F32R)
                            nc.tensor.transpose(
                                tp[:, 0:128],
                                vst[:, st4 * 128 : (st4 + 1) * 128],
                                id_sb[:],
                            )
                            nc.vector.tensor_copy(va[:, st, 0:64], tp[:, 0:64])
                            nc.scalar.copy(va[:, st, 128:192], tp[:, 64:128])

                    def emit_kt(kt):
                        nj = min(3, N_ST - kt)
                        W = nj * 128
                        q0 = kt * 128
                        # scoresT both heads (fp32r, row-tiled stationaries)
                        sps = pssc.tile([128, 2, 512], F32, tag="tsc")
                        for h in (0, 1):
                            nc.tensor.matmul(
                                sps[:, h, 0:W],
                                kt_sb[h * 64 : (h + 1) * 64, q0 : q0 + 128],
                                qt_sb[h * 64 : (h + 1) * 64, q0 : q0 + W],
                                start=True,
                                stop=False,
                            )
                        # band mask add: m3 = [tri_a | 0 | tri_b]. For the full
                        # window only slices {0,2} need masking -> strided
                        # [2,128] matmul (FD=256 keeps fp32r at 1 cyc/row).
                        for h in (0, 1):
                            if nj == 3:
                                sh = sps[:, h, :].rearrange("p (t c) -> p t c", c=128)
                                mh = m3_sb.rearrange("p (t c) -> p t c", c=128)
                                nc.tensor.matmul(
                                    sh[:, 0::2, :],
                                    id_sb[:],
                                    mh[:, 0::2, :],
                                    start=False,
                                    stop=True,
                                )
                            else:
                                nc.tensor.matmul(
                                    sps[:, h, 0:W],
                                    id_sb[:],
                                    m3_sb[:, 0:W],
                                    start=False,
                                    stop=True,
                                )
                        # exp over both heads in one ACT op
                        ex = expp.tile([128, 2, 384], BF16)
                        nc.scalar.activation(
                            ex[:, :, 0:W],
                            sps[:, :, 0:W],
                            mybir.ActivationFunctionType.Exp,
                        )
                        # ctx + dn; q-tile pair (2m, 2m+1) shares one bank:
                        # [ctx_even | dn_even | ctx_odd | dn_odd] x 128
                        for j in range(nj):
                            qt = kt + j
                            m = qt // 2
                            if m not in cd_tiles:
                                cd_tiles[m] = pscd.tile(
                                    [128, 512], F32, tag="tcd", name=f"cd{m}"
                                )
                        for grp in ("ctx", "dn"):
                            for j in range(nj):
                                qt = kt + j
                                m = qt // 2
                                off = (qt % 2) * 256
                                dofs = 0 if grp == "ctx" else 128
                                cd = cd_tiles[m]
                                first = (
                                    grp == "ctx"
                                    and qt == 2 * m
                                    and kt == max(2 * m - 2, 0)
                                )
                                last = grp == "dn" and qt == 2 * m + 1 and kt == qt
                                for h in (0, 1):
                                    if grp == "ctx":
                                        lhsT = va[:, kt, h * 64 : h * 64 + 128]
                                    else:
                                        lhsT = on2[:, h * 64 : h * 64 + 128]
                                    nc.tensor.matmul(
                                        cd[:, off + dofs : off + dofs + 128],
                                        lhsT,
                                        ex[:, h, j * 128 : (j + 1) * 128],
                                        start=(first and h == 0),
                                        stop=(last and h == 1),
                                        skip_group_check=True,
                                    )
                        # finalize q-tile qt == kt
                        qt = kt
                        m = qt // 2
                        off = (qt % 2) * 256
                        cd = cd_tiles[m]
                        rec = recp.tile([128, 128], F32)
                        nc.vector.reciprocal(rec[:], cd[:, off + 128 : off + 256])
                        ctxn = ctxp.tile([128, 128], BF16)
                        nc.vector.tensor_mul(ctxn[:], cd[:, off : off + 128], rec[:])
                        if qt % 2 == 1 or qt == N_ST - 1:
                            del cd_tiles[m]
                        # out-projection + bias; both halves in one psum tile
                        osb = outp.tile([128, H], BF16)
                        ops = pssc.tile([128, 2, 512], F32, tag="tsc")
                        nc.tensor.matmul(
                            ops[:, 0, :], ctxn[:], wo_sb[:, 0:512], start=True, stop=True
                        )
                        nc.tensor.matmul(
                            ops[:, 1, :],
                            ctxn[:],
                            wo_sb[:, 512:1024],
                            start=True,
                            stop=True,
                        )
                        nc.vector.tensor_add(osb[:, 0:512], ops[:, 0, :], bo_sb[:, 0:512])
                        nc.scalar.copy(osb[:, 512:1024], ops[:, 1, :])
                        nc.gpsimd.tensor_add(
                            osb[:, 512:1024], osb[:, 512:1024], bo_sb[:, 512:1024]
                        )
                        out_eng = nc.gpsimd if out_q == "gpsimd" else nc.sync
                        out_eng.dma_start(
                            out=out_ap[qt * 128 : (qt + 1) * 128, :], in_=osb[:]
                        )

                    # interleaved emission: chunk sc, then kts 4sc-2..4sc+1
                    for sc in range(N_SC):
                        emit_chunk(sc)
                        for kt in range(max(4 * sc - 2, 0), 4 * sc + 2):
                            emit_kt(kt)
                    for kt in (N_ST - 2, N_ST - 1):
                        emit_kt(kt)

    nc.compile()
    return nc


def build_in_maps(x, wq, wk, wv, wo, bo):
    xf = np.asarray(x, dtype=np.float32).reshape(S, H)
    # xT blocked: (p, kt, s) = x[s, kt*128 + p]
    xT_b = np.ascontiguousarray(xf.reshape(S, N_KT, 128).transpose(2, 1, 0))

    b = np.arange(128)[:, None]  # k within tile
    a = np.arange(128)[None, :]  # q within tile
    mask_a = np.where(b <= a, 0.0, NEG).astype(np.float32)  # diag tile (qt==kt)
    mask_b = np.where(b > a, 0.0, NEG).astype(np.float32)  # qt==kt+2 tile
    m3 = np.ascontiguousarray(
        np.concatenate([mask_a, np.zeros((128, 128), np.float32), mask_b], axis=1)
    )
    ident = np.eye(128, dtype=np.float32)

    def blk(wT):  # [H, CD] -> [128, N_KT, CD] float32
        return np.ascontiguousarray(
            wT.reshape(N_KT, 128, CD).transpose(1, 0, 2).astype(np.float32)
        )

    in_maps = []
    for c in range(N_CORES):
        r0, r1 = c * CD, (c + 1) * CD
        bo_c = (bo if c == 0 else np.zeros_like(bo)).astype(np.float32)
        bo_b = np.ascontiguousarray(np.broadcast_to(bo_c, (128, H)).astype(BF))
        in_maps.append(
            {
                "xT_b": xT_b,
                "wq_b": blk(np.asarray(wq, np.float32)[r0:r1, :].T),
                "wk_b": blk(np.asarray(wk, np.float32)[r0:r1, :].T),
                "wv_b": blk(np.asarray(wv, np.float32)[r0:r1, :].T),
                "wo_b": np.ascontiguousarray(
                    np.asarray(wo, np.float32)[:, r0:r1].T.astype(BF)
                ),
                "bo_b": bo_b,
                "m3_b": m3,
                "id_b": ident,
            }
        )
    return in_maps


_NC_CACHE = None


def kernel(x, wq, wk, wv, wo, bo):
    global _NC_CACHE
    if _NC_CACHE is None:
        _NC_CACHE = build_program()
    nc = _NC_CACHE
    in_maps = build_in_maps(x, wq, wk, wv, wo, bo)
    res = run_bass_kernel_spmd(nc, in_maps, list(range(N_CORES)))
    out = res.results[0]["out"].astype(np.float64)
    for c in range(1, N_CORES):
        out += res.results[c]["out"].astype(np.float64)
    return out.reshape(1, S, H).astype(np.float32)


# revision 25
# speedup vs baseline: 1.2589x; 1.1546x over previous
"""Trainium2 Bass kernel for sliding-window (window=256) causal attention.

Model (B=1, S=4096, H=1024, nh=16, hd=64, no q-scaling):
  q,k,v = x@wq.T, x@wk.T, x@wv.T ; scores = q@k.T (banded causal window 256)
  out = softmax(scores)@v reassembled, then @wo.T + bo

Sharding: 2 heads per core across 8 cores (tensor parallel on the head dim).
Each core computes a partial output  ctx_c @ wo[:, c-slice].T  (+ bias on
core 0 only); the host sums the 8 bf16 partials (the all-reduce step).

v3 (fp32 q/k/scores for precision, bf16 attention weights + out path,
x pre-transposed on host, phases interleaved per 512-column chunk):
  chunk sc: DMA xT chunk (fp32r); QT/KT = w.T.T @ xT (fp32r, FD=512);
            VT likewise, then PE-transpose 128-tiles -> va bf16 padded
            [V_h0 | 0 | V_h1] (per k-tile [s, d] orientation).
  key-tile kt (4 per chunk, interleaved):
      scoresT[k,q] h0/h1 fp32r (row-tiled stationaries) into a 2-bank psum
      tile; band-mask added via fp32r identity-matmul of m3=[tri|0|tri];
      one exp over both heads (ACT) -> ex bf16;
      ctx/dn accumulated per q-tile with padded bf16 stationaries; ctx+dn
      of two consecutive q-tiles share one psum bank (single accumulation
      group per bank, per-element first-write-overwrite semantics);
      normalize with reciprocal+mul (DVE, direct from psum);
      out-projection (bf16, FD=512 x2 into one 2-bank psum tile) + bias:
      half0 fused into the DVE psum->sbuf add, half1 via ACT copy +
      GPSIMD bias add; DMA out bf16.

PSUM budget (8 banks): tsc pool [128,2,512] bufs=3 (6 banks; scores +
out-proj share) + tcd pool [128,512] bufs=2 (2 banks; q/k/v psums,
transpose psums, and ctx/dn pair banks share).
"""

import numpy as np
import ml_dtypes

import concourse.bass as bass
import concourse.tile as tile
from concourse import bacc, mybir
from concourse.bass_utils import run_bass_kernel_spmd

S = 4096
H = 1024
NH = 16
HD = 64
WIN = 256
N_CORES = 8
HEADS_PER_CORE = NH // N_CORES  # 2
CD = HEADS_PER_CORE * HD  # 128 ctx dims per core
NEG = -1e30

F32 = mybir.dt.float32
F32R = mybir.dt.float32r
BF16 = mybir.dt.bfloat16

N_ST = S // 128  # 32 s-tiles
N_KT = H // 128  # 8 contraction tiles for projections
N_SC = S // 512  # 8 s-chunks for projections
BF = ml_dtypes.bfloat16


def build_program(taps=False, reps=1, chunk_in_tsc=False, out_q="sync",
                  tsc_bufs=3, tcd_bufs=2, xst_bufs=2, expp_bufs=3,
                  outp_bufs=3, ctxp_bufs=3):
    nc = bacc.Bacc("TRN2", target_bir_lowering=False, debug=False)

    xT_ap = nc.dram_tensor("xT_b", [128, N_KT, S], F32R, kind="ExternalInput").ap()
    wq_ap = nc.dram_tensor("wq_b", [128, N_KT, CD], F32R, kind="ExternalInput").ap()
    wk_ap = nc.dram_tensor("wk_b", [128, N_KT, CD], F32R, kind="ExternalInput").ap()
    wv_ap = nc.dram_tensor("wv_b", [128, N_KT, CD], F32R, kind="ExternalInput").ap()
    wo_ap = nc.dram_tensor("wo_b", [CD, H], BF16, kind="ExternalInput").ap()
    bo_ap = nc.dram_tensor("bo_b", [128, H], BF16, kind="ExternalInput").ap()
    m3_ap = nc.dram_tensor("m3_b", [128, 384], F32R, kind="ExternalInput").ap()
    id_ap = nc.dram_tensor("id_b", [128, 128], F32R, kind="ExternalInput").ap()
    out_ap = nc.dram_tensor("out", [S, H], BF16, kind="ExternalOutput").ap()

    with tile.TileContext(nc) as tc:
        with (
            tc.tile_pool(name="consts", bufs=1) as consts,
            tc.tile_pool(name="big", bufs=1) as big,
        ):
            # ---- constant loads ----
            wq_sb = consts.tile([128, N_KT, CD], F32R)
            wk_sb = consts.tile([128, N_KT, CD], F32R)
            wv_sb = consts.tile([128, N_KT, CD], F32R)
            for w_sb, w_ap in ((wq_sb, wq_ap), (wk_sb, wk_ap), (wv_sb, wv_ap)):
                nc.sync.dma_start(out=w_sb[:], in_=w_ap[:])
            wo_sb = consts.tile([128, H], BF16)
            nc.sync.dma_start(out=wo_sb[:], in_=wo_ap[:])
            bo_sb = consts.tile([128, H], BF16)
            nc.sync.dma_start(out=bo_sb[:], in_=bo_ap[:])
            m3_sb = consts.tile([128, 384], F32R)
            nc.sync.dma_start(out=m3_sb[:], in_=m3_ap[:])
            id_sb = consts.tile([128, 128], F32R)
            nc.sync.dma_start(out=id_sb[:], in_=id_ap[:])
            # dn stationaries: [ones | zeros | ones]; h0 -> 0:128, h1 -> 64:192
            on2 = consts.tile([128, 192], BF16)
            nc.gpsimd.memset(on2[:, 0:64], 1.0)
            nc.gpsimd.memset(on2[:, 64:128], 0.0)
            nc.gpsimd.memset(on2[:, 128:192], 1.0)

            # ---- persistent activations ----
            qt_sb = big.tile([128, S], F32R)  # QT: [2h*64 dims, S]
            kt_sb = big.tile([128, S], F32R)
            # V per k-tile, padded: [V_h0(64) | zeros(64) | V_h1(64)]
            va = big.tile([128, N_ST, 192], BF16)
            nc.gpsimd.memset(va[:, :, 64:128], 0.0)

            for _rep in range(reps):
                with (
                    tc.tile_pool(name="xst", bufs=xst_bufs) as xst_p,
                    tc.tile_pool(name="vst", bufs=2) as vst_p,
                    tc.tile_pool(name="pssc", bufs=tsc_bufs, space="PSUM") as pssc,
                    tc.tile_pool(name="pscd", bufs=tcd_bufs, space="PSUM") as pscd,
                    tc.tile_pool(name="expp", bufs=expp_bufs) as expp,
                    tc.tile_pool(name="recp", bufs=3) as recp,
                    tc.tile_pool(name="ctxp", bufs=ctxp_bufs) as ctxp,
                    tc.tile_pool(name="outp", bufs=outp_bufs) as outp,
                ):
                    cd_tiles = {}  # pair index -> psum tile [128, 512]
                    cp_state = [0]

                    def chunk_ps(name):
                        # [128,512] psum for phase-1; lives in the tsc pool
                        # (short-lived neighbors) or the tcd pool.
                        if chunk_in_tsc:
                            t = pssc.tile([128, 2, 512], F32, tag="tsc", name=name)
                            return t[:, 0, :]
                        return pscd.tile([128, 512], F32, tag="tcd", name=name)

                    def emit_chunk(sc):
                        s0 = sc * 512
                        xst = xst_p.tile([128, N_KT, 512], F32R, tag="xst", name="xst")
                        x_eng = nc.sync if sc % 2 == 0 else nc.scalar
                        x_eng.dma_start(out=xst[:], in_=xT_ap[:, :, s0 : s0 + 512])
                        for w_sb, dstT in ((wq_sb, qt_sb), (wk_sb, kt_sb)):
                            pps = chunk_ps("pps")
                            for kt in range(N_KT):
                                nc.tensor.matmul(
                                    pps[:],
                                    w_sb[:, kt, :],
                                    xst[:, kt, :],
                                    start=(kt == 0),
                                    stop=(kt == N_KT - 1),
                                )
                            dst = dstT[:, s0 : s0 + 512]
                            if cp_state[0] % 2 == 0:
                                nc.vector.tensor_copy(dst, pps[:])
                            else:
                                nc.scalar.copy(dst, pps[:])
                            cp_state[0] += 1
                        # V: VT chunk then PE-transpose each 128-tile
                        vps = chunk_ps("vps")
                        for kt in range(N_KT):
                            nc.tensor.matmul(
                                vps[:],
                                wv_sb[:, kt, :],
                                xst[:, kt, :],
                                start=(kt == 0),
                                stop=(kt == N_KT - 1),
                            )
                        vst = vst_p.tile([128, 512], F32R, tag="vst", name="vst")
                        if cp_state[0] % 2 == 0:
                            nc.vector.tensor_copy(vst[:], vps[:])
                        else:
                            nc.scalar.copy(vst[:], vps[:])
                        cp_state[0] += 1
                        for st4 in range(4):
                            st = sc * 4 + st4
                            tp = chunk_ps("tp").bitcast(F32R)
                            nc.tensor.transpose(
                                tp[:, 0:128],
                                vst[:, st4 * 128 : (st4 + 1) * 128],
                                id_sb[:],
                            )
                            nc.vector.tensor_copy(va[:, st, 0:64], tp[:, 0:64])
                            nc.scalar.copy(va[:, st, 128:192], tp[:, 64:128])

                    def emit_kt(kt):
                        nj = min(3, N_ST - kt)
                        W = nj * 128
                        q0 = kt * 128
                        # scoresT both heads (fp32r, row-tiled stationaries)
                        sps = pssc.tile([128, 2, 512], F32, tag="tsc")
                        for h in (0, 1):
                            nc.tensor.matmul(
                                sps[:, h, 0:W],
                                kt_sb[h * 64 : (h + 1) * 64, q0 : q0 + 128],
                                qt_sb[h * 64 : (h + 1) * 64, q0 : q0 + W],
                                start=True,
                                stop=False,
                            )
                        # band mask add: m3 = [tri_a | 0 | tri_b]. For the full
                        # window only slices {0,2} need masking -> strided
                        # [2,128] matmul (FD=256 keeps fp32r at 1 cyc/row).
                        for h in (0, 1):
                            if nj == 3:
                                sh = sps[:, h, :].rearrange("p (t c) -> p t c", c=128)
                                mh = m3_sb.rearrange("p (t c) -> p t c", c=128)
                                nc.tensor.matmul(
                                    sh[:, 0::2, :],
                                    id_sb[:],
                                    mh[:, 0::2, :],
                                    start=False,
                                    stop=True,
                                )
                            else:
                                nc.tensor.matmul(
                                    sps[:, h, 0:W],
                                    id_sb[:],
                                    m3_sb[:, 0:W],
                                    start=False,
                                    stop=True,
                                )
                        # exp over both heads in one ACT op
                        ex = expp.tile([128, 2, 384], BF16)
                        nc.scalar.activation(
                            ex[:, :, 0:W],
                            sps[:, :, 0:W],
                            mybir.ActivationFunctionType.Exp,
                        )
                        # ctx + dn; q-tile pair (2m, 2m+1) shares one bank:
                        # [ctx_even | dn_even | ctx_odd | dn_odd] x 128
                        for j in range(nj):
                            qt = kt + j
                            m = qt // 2
                            if m not in cd_tiles:
                                cd_tiles[m] = pscd.tile(
                                    [128, 512], F32, tag="tcd", name=f"cd{m}"
                                )
                        # Build per-kt work groups: adjacent same-bank q-tiles
                        # (even qt, qt+1) merge into one strided matmul.
                        # groups: (j0, njj, first, last); njj in {1, 2}
                        groups = []
                        j = 0
                        while j < nj:
                            qt = kt + j
                            m = qt // 2
                            if qt % 2 == 0 and j + 1 < nj:
                                groups.append((j, 2, kt == 0 and m == 0, False))
                                j += 2
                            else:
                                first = qt == 2 * m and kt == max(2 * m - 2, 0)
                                last = qt == 2 * m + 1 and kt == qt
                                groups.append((j, 1, first, last))
                                j += 1
                        for grp in ("ctx", "dn"):
                            dofs = 0 if grp == "ctx" else 128
                            for j0, njj, first, last in groups:
                                first = first and grp == "ctx"
                                last = last and grp == "dn"
                                qt = kt + j0
                                cd = cd_tiles[qt // 2]
                                if njj == 2:
                                    cdr = cd.rearrange("p (t c) -> p t c", c=128)
                                    out_ap_ = cdr[:, (dofs // 128) :: 2, :]
                                else:
                                    off = (qt % 2) * 256
                                    out_ap_ = cd[:, off + dofs : off + dofs + 128]
                                for h in (0, 1):
                                    if grp == "ctx":
                                        lhsT = va[:, kt, h * 64 : h * 64 + 128]
                                    else:
                                        lhsT = on2[:, h * 64 : h * 64 + 128]
                                    nc.tensor.matmul(
                                        out_ap_,
                                        lhsT,
                                        ex[:, h, j0 * 128 : (j0 + njj) * 128],
                                        start=(first and h == 0),
                                        stop=(last and h == 1),
                                        skip_group_check=True,
                                    )
                        # finalize q-tile qt == kt
                        qt = kt
                        m = qt // 2
                        off = (qt % 2) * 256
                        cd = cd_tiles[m]
                        rec = recp.tile([128, 128], F32)
                        nc.vector.reciprocal(rec[:], cd[:, off + 128 : off + 256])
                        ctxn = ctxp.tile([128, 128], BF16)
                        nc.vector.tensor_mul(ctxn[:], cd[:, off : off + 128], rec[:])
                        if qt % 2 == 1 or qt == N_ST - 1:
                            del cd_tiles[m]
                        # out-projection + bias; both halves in one psum tile
                        osb = outp.tile([128, H], BF16)
                        ops = pssc.tile([128, 2, 512], F32, tag="tsc")
                        nc.tensor.matmul(
                            ops[:, 0, :], ctxn[:], wo_sb[:, 0:512], start=True, stop=True
                        )
                        nc.tensor.matmul(
                            ops[:, 1, :],
                            ctxn[:],
                            wo_sb[:, 512:1024],
                            start=True,
                            stop=True,
                        )
                        nc.vector.tensor_add(osb[:, 0:512], ops[:, 0, :], bo_sb[:, 0:512])
                        nc.scalar.copy(osb[:, 512:1024], ops[:, 1, :])
                        nc.gpsimd.tensor_add(
                            osb[:, 512:1024], osb[:, 512:1024], bo_sb[:, 512:1024]
                        )
                        out_eng = nc.gpsimd if out_q == "gpsimd" else nc.sync
                        out_eng.dma_start(
                            out=out_ap[qt * 128 : (qt + 1) * 128, :], in_=osb[:]
                        )

                    # interleaved emission: chunk sc, then kts 4sc-2..4sc+1
                    for sc in range(N_SC):
                        emit_chunk(sc)
                        for kt in range(max(4 * sc - 2, 0), 4 * sc + 2):
                            emit_kt(kt)
                    for kt in (N_ST - 2, N_ST - 1):
                        emit_kt(kt)

    nc.compile()
    return nc


def build_in_maps(x, wq, wk, wv, wo, bo):
    xf = np.asarray(x, dtype=np.float32).reshape(S, H)
    # xT blocked: (p, kt, s) = x[s, kt*128 + p]
    xT_b = np.ascontiguousarray(xf.reshape(S, N_KT, 128).transpose(2, 1, 0))

    b = np.arange(128)[:, None]  # k within tile
    a = np.arange(128)[None, :]  # q within tile
    mask_a = np.where(b <= a, 0.0, NEG).astype(np.float32)  # diag tile (qt==kt)
    mask_b = np.where(b > a, 0.0, NEG).astype(np.float32)  # qt==kt+2 tile
    m3 = np.ascontiguousarray(
        np.concatenate([mask_a, np.zeros((128, 128), np.float32), mask_b], axis=1)
    )
    ident = np.eye(128, dtype=np.float32)

    def blk(wT):  # [H, CD] -> [128, N_KT, CD] float32
        return np.ascontiguousarray(
            wT.reshape(N_KT, 128, CD).transpose(1, 0, 2).astype(np.float32)
        )

    in_maps = []
    for c in range(N_CORES):
        r0, r1 = c * CD, (c + 1) * CD
        bo_c = (bo if c == 0 else np.zeros_like(bo)).astype(np.float32)
        bo_b = np.ascontiguousarray(np.broadcast_to(bo_c, (128, H)).astype(BF))
        in_maps.append(
            {
                "xT_b": xT_b,
                "wq_b": blk(np.asarray(wq, np.float32)[r0:r1, :].T),
                "wk_b": blk(np.asarray(wk, np.float32)[r0:r1, :].T),
                "wv_b": blk(np.asarray(wv, np.float32)[r0:r1, :].T),
                "wo_b": np.ascontiguousarray(
                    np.asarray(wo, np.float32)[:, r0:r1].T.astype(BF)
                ),
                "bo_b": bo_b,
                "m3_b": m3,
                "id_b": ident,
            }
        )
    return in_maps


_NC_CACHE = None


def kernel(x, wq, wk, wv, wo, bo):
    global _NC_CACHE
    if _NC_CACHE is None:
        _NC_CACHE = build_program()
    nc = _NC_CACHE
    in_maps = build_in_maps(x, wq, wk, wv, wo, bo)
    res = run_bass_kernel_spmd(nc, in_maps, list(range(N_CORES)))
    out = res.results[0]["out"].astype(np.float64)
    for c in range(1, N_CORES):
        out += res.results[c]["out"].astype(np.float64)
    return out.reshape(1, S, H).astype(np.float32)
